# revision 28
# baseline (speedup 1.0000x reference)
"""Trainium2 Bass kernel for BasisFunction1D (piecewise-linear basis / histogram binning).

Math:
  out[o, b] = sum_i (1-d)*P[g, o, i] + d*P[g+1, o, i],
  g = bucket of x[i,b] on a Laplace-CDF grid, d = in-bucket linear position.

Key identity: with u = g + d the per-dim function is piecewise-linear in u with
knots at the integers, so using hat(v) = relu(v+1) - 2 relu(v) + relu(v-1):

  sum_g P_g * hat(u - g)  =  P_0 * (u+1)  +  sum_m Q_m * relu(u - m)

with Q_m = P_{m+1} - 2 P_m + P_{m-1} (second difference along g, built on host).
This needs ONE fused elementwise op per grid cell instead of the two-pass
|.| / min hat construction, and no per-input-dim PE broadcast at all.

fp32r matmul inputs are rounded to a ~10-bit mantissa, so raw relu(u-m) values
(up to 128) lose too much precision.  Two-level fix keeping every rhs in [0,8]:
with c~_q = min(u - 8q, 8)  (16 coarse tensors, one tensor_scalar each),

  relu(u - 8q - j) = max(c~_q - j, 0) + relu(u - 8(q+1))
  relu(u - 8k)     = sum_{k'>=k} max(c~_k', 0) + relu(u - 128)
  u                = sum_k max(c~_k, 0) + relu(u - 128) - relu(-u)

Folding the telescoped coarse sums into the host-side lhs table:

  out = C*1 + sum_{q,j} L_{q,j} * max(c~_q - j, 0)
        [+ OV * relu(u - 128) + RL * relu(-u)]   (zero on the closed-form path)

  L_{q,j} = Q_{8q+j} + (j==0) * (S'_q + P_0),  S'_q = sum_{q'<q} sum_j Q_{8q'+j}
  C = P_0,  OV = sum_m Q_m + P_0,  RL = P_0 - P_1

Every fine tile is ONE fused tensor_scalar / activation instruction, all rhs
values lie in [0,8] (exactly representable saturations; only the active
block's fractional value rounds), and all matmuls run fp32r at 1 cycle/row.

Device algorithm (per core, batch shard of 1024):
  1. Compute u[i,b] = g + d in closed form (the borders array is the
     inverse-Laplace-CDF grid; verified host-side, with an exact host-table
     fallback otherwise).
  2. ones tile (memset), then per q: c~_q on DVE, then 8 fine tiles
     round-robin on DVE / ACT / Pool so no single engine bottlenecks.
  3. out accumulates in PSUM over the matching fp32r matmuls against the
     host-side table [C, L_0..L_127, OV, RL].
"""

import math

import numpy as np

I_DIM = 128
O_DIM = 128
G = 128
B_FULL = 8192
N_CORES = 8
BS = B_FULL // N_CORES
NBLK = 131  # C(ones), L_0..L_127, OV, RL

_NC_CACHE = {}


def _ref_grid_f64():
    def inv(u):
        return math.log(2.0 * u) if u <= 0.5 else -math.log(2.0 * (1.0 - u))

    cs = 1.0 / G
    b = [inv(i * cs) for i in range(1, G)]
    left = b[0] - (b[1] - b[0])
    right = b[-1] + (b[-1] - b[-2])
    return np.array([left] + b + [right], dtype=np.float64)


def _grid_matches(borders, inv_len):
    ref = _ref_grid_f64()
    ref32 = ref.astype(np.float32)
    il_ref = (1.0 / (ref32[1:].astype(np.float64) - ref32[:-1].astype(np.float64))).astype(
        np.float32
    )
    return np.allclose(borders, ref32, rtol=1e-5, atol=1e-5) and np.allclose(
        inv_len, il_ref, rtol=1e-4, atol=1e-4
    )


def _build_nc(host_tables: bool, reps: int = 1):
    from contextlib import ExitStack

    import concourse.bacc as bacc
    import concourse.mybir as mybir
    import concourse.tile as tile

    dt = mybir.dt
    f32 = dt.float32
    f32r = dt.float32r
    AF = mybir.ActivationFunctionType
    OP = mybir.AluOpType

    nc = bacc.Bacc("TRN2", target_bir_lowering=False, debug=False)

    x_d = nc.dram_tensor("x", [I_DIM, BS], f32, kind="ExternalInput")
    # [i, j*o]: block j=0 is P_0^T (linear term), j=1..128 are Q_{j-1}^T,
    # j=129 is (2P_0 - P_1)^T (u<0 correction)
    qt_d = nc.dram_tensor("qt", [I_DIM, NBLK * O_DIM], f32r, kind="ExternalInput")
    # bias table: col m (0..127) = -m (ACT relu biases); col 128 = -64; col 129 = -63.5
    nb_d = nc.dram_tensor("nb", [128, 130], f32, kind="ExternalInput")
    if host_tables:
        hg_d = nc.dram_tensor("hg", [I_DIM, BS], f32, kind="ExternalInput")
        hb_d = nc.dram_tensor("hbor", [I_DIM, BS], f32, kind="ExternalInput")
        hi_d = nc.dram_tensor("hil", [I_DIM, BS], f32, kind="ExternalInput")
    out_d = nc.dram_tensor("out", [O_DIM, BS], f32, kind="ExternalOutput")

    with tile.TileContext(nc) as tc, ExitStack() as ctx:
        if reps > 1:
            loop_cm = tc.For_i(
                0,
                reps,
                1,
                hint_engines=(
                    mybir.EngineType.PE,
                    mybir.EngineType.Activation,
                    mybir.EngineType.DVE,
                ),
            )
            ctx.enter_context(loop_cm)
        pers = ctx.enter_context(tc.tile_pool(name="pers", bufs=1))
        scr = ctx.enter_context(tc.tile_pool(name="scr", bufs=1))
        rpool = ctx.enter_context(tc.tile_pool(name="rpool", bufs=10))
        cpool = ctx.enter_context(tc.tile_pool(name="cpool", bufs=6))
        opsum = ctx.enter_context(tc.tile_pool(name="opsum", bufs=1, space="PSUM"))

        x_sb = pers.tile([I_DIM, BS], f32, tag="x", name="x")
        nc.sync.dma_start(x_sb[:], x_d.ap())
        nb_sb = pers.tile([128, 130], f32, tag="nb", name="nb")
        nc.sync.dma_start(nb_sb[:], nb_d.ap())

        # Q table in 8 chunks so early matmuls don't wait on the full load
        NCH = 8
        qt_sb = pers.tile([I_DIM, NBLK * O_DIM], f32r, tag="qt", name="qt")
        per = (NBLK + NCH - 1) // NCH
        for c in range(NCH):
            lo = c * per * O_DIM
            hi = min(NBLK, (c + 1) * per) * O_DIM
            nc.sync.dma_start(qt_sb[:, lo:hi], qt_d.ap()[:, lo:hi])

        def sct(tag, dtype=f32):
            return scr.tile([I_DIM, BS], dtype, tag=tag, name=tag)

        u = pers.tile([I_DIM, BS], f32, tag="u", name="u")

        if not host_tables:
            # ---- closed-form u = g + (x - borders[g]) * inv_len[g] ----
            # processed in two column halves so ACT/DVE stages pipeline
            gf = pers.tile([I_DIM, BS], f32, tag="gf", name="gf")
            HB = BS // 2
            for h in range(2):
                cs = slice(h * HB, (h + 1) * HB)
                ax = sct(f"T0{h}")
                nc.scalar.activation(ax[:, cs], x_sb[:, cs], AF.Abs)
                e = sct(f"T1{h}")
                nc.scalar.activation(e[:, cs], ax[:, cs], AF.Exp, scale=-1.0)
                s = sct(f"T2{h}")
                nc.scalar.activation(s[:, cs], x_sb[:, cs], AF.Sign)
                se = sct(f"T0{h}")
                nc.gpsimd.tensor_mul(se[:, cs], s[:, cs], e[:, cs])
                t1 = sct(f"T1{h}")
                nc.vector.tensor_sub(t1[:, cs], s[:, cs], se[:, cs])
                # y = cdf * 128 = 64 + 64*s*(1-e)
                y = sct(f"T0{h}")
                nc.vector.tensor_scalar(y[:, cs], t1[:, cs], 64.0, 64.0, OP.mult, OP.add)
                gi = scr.tile([I_DIM, BS], dt.int32, tag=f"T3{h}", name=f"T3{h}")
                nc.vector.tensor_copy(gi[:, cs], y[:, cs])
                g0 = sct(f"T1{h}")
                nc.vector.tensor_copy(g0[:, cs], gi[:, cs])
                # robust floor regardless of the converter's rounding mode
                cg = sct(f"T4{h}")
                nc.vector.tensor_tensor(cg[:, cs], g0[:, cs], y[:, cs], op=OP.is_gt)
                gf1 = sct(f"T3{h}")
                nc.vector.tensor_sub(gf1[:, cs], g0[:, cs], cg[:, cs])
                nc.vector.tensor_scalar(gf[:, cs], gf1[:, cs], 0.0, 127.0, OP.max, OP.min)
                # borders[g] = sign * ln(m1/64), m1 = max(min(g, 128-g), 0.5)
                a1 = sct(f"T1{h}")
                nc.scalar.activation(a1[:, cs], gf[:, cs], AF.Abs, bias=nb_sb[:, 128:129])
                a1c = sct(f"T3{h}")
                nc.vector.tensor_scalar(a1c[:, cs], a1[:, cs], 63.5, None, OP.min)
                L = sct(f"T1{h}")
                nc.scalar.activation(L[:, cs], a1c[:, cs], AF.Ln, scale=-1.0 / 64.0, bias=1.0)
                sL = sct(f"T3{h}")
                nc.gpsimd.tensor_mul(sL[:, cs], s[:, cs], L[:, cs])
                xb = sct(f"T2{h}")  # x - borders[g]  (= x + s*L)
                nc.gpsimd.tensor_add(xb[:, cs], x_sb[:, cs], sL[:, cs])
                # inv_len[g] = 1/ln(1 + 1/m2), m2 = max(min(g, 127-g), 1)
                a2 = sct(f"T0{h}")
                nc.scalar.activation(a2[:, cs], gf[:, cs], AF.Abs, bias=nb_sb[:, 129:130])
                nm2 = sct(f"T1{h}")  # -m2
                nc.vector.tensor_scalar(nm2[:, cs], a2[:, cs], 63.5, -1.0, OP.subtract, OP.min)
                rm = sct(f"T0{h}")  # -1/m2
                nc.vector.reciprocal_approx_fast(rm[:, cs], nm2[:, cs])
                q = sct(f"T1{h}")  # ln(1 + 1/m2)
                nc.scalar.activation(q[:, cs], rm[:, cs], AF.Ln, scale=-1.0, bias=1.0)
                il = sct(f"T0{h}")
                nc.vector.reciprocal_approx_fast(il[:, cs], q[:, cs])
                d_ = sct(f"T1{h}")
                nc.vector.tensor_mul(d_[:, cs], xb[:, cs], il[:, cs])
                nc.vector.tensor_add(u[:, cs], gf[:, cs], d_[:, cs])
        else:
            hg_sb = pers.tile([I_DIM, BS], f32, tag="hg", name="hg")
            nc.sync.dma_start(hg_sb[:], hg_d.ap())
            hb_sb = sct("T0")
            nc.sync.dma_start(hb_sb[:], hb_d.ap())
            hi_sb = sct("T1")
            nc.sync.dma_start(hi_sb[:], hi_d.ap())
            xb = sct("T2")
            nc.vector.tensor_sub(xb[:], x_sb[:], hb_sb[:])
            d_ = sct("T0")
            nc.vector.tensor_mul(d_[:], xb[:], hi_sb[:])
            nc.vector.tensor_add(u[:], hg_sb[:], d_[:])

        # ---- main loop: rhs tile then its two accumulate matmuls ----
        # block 0: ones; blocks 1+8q+j: max(c~_q - j, 0); 129: relu(u-128),
        # 130: relu(-u).  Fine producers round-robin DVE/ACT (gpsimd
        # tensor_scalar is uncalibrated software); consecutive blocks
        # alternate between two PSUM regions so back-to-back accumulates
        # never target the same bank; the two regions are summed at the end.
        HALF = BS // 2
        acc0 = opsum.tile([O_DIM, HALF], f32, tag="acc0", name="acc0")
        acc1 = opsum.tile([O_DIM, HALF], f32, tag="acc1", name="acc1")

        def mm_pair(blk, t):
            # column halves go to different PSUM bank regions so consecutive
            # matmuls never accumulate into the same bank back-to-back
            first = blk == 0
            last = blk == NBLK - 1
            lhsT = qt_sb[:, blk * O_DIM : (blk + 1) * O_DIM]
            nc.tensor.matmul(
                acc0[:, :], lhsT, t[:, 0:HALF],
                start=first, stop=last, skip_group_check=True,
            )
            nc.tensor.matmul(
                acc1[:, :], lhsT, t[:, HALF:BS],
                start=first, stop=last, skip_group_check=True,
            )

        ones = rpool.tile([I_DIM, BS], f32r, tag="r", name="r")
        nc.vector.tensor_scalar(ones[:], x_sb[:], 0.0, 1.0, OP.mult, OP.add)
        mm_pair(0, ones)

        # deterministic weighted assignment of the 128 fine tiles
        share = {"D": 60.0 / 128, "A": 46.0 / 128, "P": 22.0 / 128}
        used = {"D": 0, "A": 0, "P": 0}
        for q in range(16):
            ct = cpool.tile([I_DIM, BS], f32, tag="c", name="c")
            nc.vector.tensor_scalar(ct[:], u[:], 8.0 * q, 8.0, OP.subtract, OP.min)
            for j in range(8):
                f = 8 * q + j
                eng = max("DAP", key=lambda e: share[e] * (f + 1) - used[e])
                used[eng] += 1
                t = rpool.tile([I_DIM, BS], f32r, tag="r", name="r")
                if eng == "A":
                    nc.scalar.activation(t[:], ct[:], AF.Relu, bias=nb_sb[:, j : j + 1])
                elif eng == "P":
                    nc.gpsimd.tensor_scalar(t[:], ct[:], float(j), 0.0, OP.subtract, OP.max)
                else:
                    nc.vector.tensor_scalar(t[:], ct[:], float(j), 0.0, OP.subtract, OP.max)
                mm_pair(1 + f, t)
        # u < 0 and u > 128 occur even on the closed-form path (the two
        # extrapolation buckets reach past the borders array)
        ov = rpool.tile([I_DIM, BS], f32r, tag="r", name="r")
        nc.vector.tensor_scalar(ov[:], u[:], 128.0, 0.0, OP.subtract, OP.max)
        mm_pair(129, ov)
        rl = rpool.tile([I_DIM, BS], f32r, tag="r", name="r")
        nc.vector.tensor_scalar(rl[:], u[:], -1.0, 0.0, OP.mult, OP.max)
        mm_pair(130, rl)

        out_sb = pers.tile([O_DIM, BS], f32, tag="osb", name="osb")
        nc.vector.tensor_copy(out_sb[:, 0:HALF], acc0[:, :])
        nc.scalar.copy(out_sb[:, HALF:BS], acc1[:, :])
        nc.sync.dma_start(out_d.ap(), out_sb[:])

    return nc


def _get_nc(host_tables: bool, reps: int = 1):
    key = (bool(host_tables), reps)
    if key not in _NC_CACHE:
        _NC_CACHE[key] = _build_nc(key[0], reps)
    return _NC_CACHE[key]


def _host_inputs(x, func_parameter, borders, inverse_chunk_lengths):
    x = np.ascontiguousarray(np.asarray(x, dtype=np.float32))
    P = np.asarray(func_parameter, dtype=np.float32)
    borders = np.asarray(borders, dtype=np.float32)
    inv_len = np.asarray(inverse_chunk_lengths, dtype=np.float32)

    host_tables = not _grid_matches(borders, inv_len)

    # blocks in use order: C(ones), L_0..L_127, OV, RL   (each [O, I])
    Pd = P.astype(np.float64)
    Qm = Pd[1 : G + 1] - 2.0 * Pd[0:G]
    Qm[1:] += Pd[0 : G - 1]  # Q_m = P_{m+1} - 2 P_m + P_{m-1}
    R = Qm.reshape(16, 8, O_DIM, I_DIM).sum(axis=1)  # [16, O, I]
    Sp = np.cumsum(R, axis=0) - R  # S'_q = sum_{q'<q} R_{q'}
    L = Qm.reshape(16, 8, O_DIM, I_DIM).copy()
    L[:, 0] += Sp + Pd[0]
    blocks = np.concatenate(
        [
            Pd[0:1],  # C
            L.reshape(G, O_DIM, I_DIM),
            (Qm.sum(axis=0) + Pd[0])[None],  # OV
            (Pd[0] - Pd[1])[None],  # RL
        ],
        axis=0,
    ).astype(np.float32)  # [131, O, I]
    qt = np.ascontiguousarray(blocks.transpose(2, 0, 1).reshape(I_DIM, NBLK * O_DIM))

    nb = np.zeros((128, 130), dtype=np.float32)
    nb[:, 0:128] = -np.arange(128, dtype=np.float32)[None, :]
    nb[:, 128] = -64.0
    nb[:, 129] = -63.5

    in_maps = []
    for c in range(N_CORES):
        xs = np.ascontiguousarray(x[:, c * BS : (c + 1) * BS])
        m = {"x": xs, "qt": qt, "nb": nb}
        if host_tables:
            # exact fallback: bucketize host-side with the provided tables
            exp_na = np.exp(-np.abs(xs))
            cdf = np.where(xs > 0, 1.0 - 0.5 * exp_na, 0.5 * exp_na).astype(np.float32)
            idx = np.clip((cdf * G).astype(np.int32), 0, G - 1)
            m["hg"] = idx.astype(np.float32)
            m["hbor"] = borders[idx].astype(np.float32)
            m["hil"] = inv_len[idx].astype(np.float32)
        in_maps.append(m)
    return in_maps, host_tables


_RUNNER_CACHE = {}


def _get_runner(host_tables, reps: int = 1):
    """Cached jitted 8-core runner (mirrors bass2jax.run_bass_via_pjrt multi-core path)."""
    key = (bool(host_tables), reps)
    if key in _RUNNER_CACHE:
        return _RUNNER_CACHE[key]

    import jax
    from jax.sharding import Mesh, PartitionSpec
    from jax.experimental.shard_map import shard_map
    import concourse.mybir as mybir
    from concourse.bass2jax import (
        _bass_exec_p,
        install_neuronx_cc_hook,
        partition_id_tensor,
    )

    install_neuronx_cc_hook()
    nc = _get_nc(host_tables, reps)
    if not nc.is_finalized():
        nc.finalize()
    assert nc.dbg_addr is None
    partition_name = nc.partition_id_tensor.name if nc.partition_id_tensor else None

    in_names, out_names, out_avals, zero_outs = [], [], [], []
    for alloc in nc.m.functions[0].allocations:
        if not isinstance(alloc, mybir.MemoryLocationSet):
            continue
        name = alloc.memorylocations[0].name
        if alloc.kind == "ExternalInput":
            if name != partition_name:
                in_names.append(name)
        elif alloc.kind == "ExternalOutput":
            shape = tuple(alloc.tensor_shape)
            dtype = mybir.dt.np(alloc.dtype)
            out_names.append(name)
            out_avals.append(jax.core.ShapedArray(shape, dtype))
            zero_outs.append(np.zeros(shape, dtype))
    n_params = len(in_names)
    all_names = in_names + out_names
    if partition_name is not None:
        all_names = all_names + [partition_name]

    def _body(*args):
        operands = list(args)
        if partition_name is not None:
            operands.append(partition_id_tensor())
        outs = _bass_exec_p.bind(
            *operands,
            out_avals=tuple(out_avals),
            in_names=tuple(all_names),
            out_names=tuple(out_names),
            lowering_input_output_aliases=(),
            sim_require_finite=True,
            sim_require_nnan=True,
            nc=nc,
        )
        return tuple(outs)

    devices = jax.devices()[:N_CORES]
    mesh = Mesh(np.asarray(devices), ("core",))
    n_outs = len(out_names)
    sharded = jax.jit(
        shard_map(
            _body,
            mesh=mesh,
            in_specs=(PartitionSpec("core"),) * (n_params + n_outs),
            out_specs=(PartitionSpec("core"),) * n_outs,
            check_rep=False,
        ),
        keep_unused=True,
    )

    def run(in_maps):
        concat_in = [
            np.concatenate([np.asarray(m[name]) for m in in_maps], axis=0)
            for name in in_names
        ]
        concat_zero = [
            np.zeros((N_CORES * z.shape[0], *z.shape[1:]), z.dtype) for z in zero_outs
        ]
        out_arrs = sharded(*concat_in, *concat_zero)
        res = [
            {
                name: np.asarray(out_arrs[i]).reshape(N_CORES, *out_avals[i].shape)[c]
                for i, name in enumerate(out_names)
            }
            for c in range(N_CORES)
        ]
        return res, (sharded, concat_in, concat_zero)

    _RUNNER_CACHE[key] = run
    return run


def _run(in_maps, host_tables, trace=False):
    run = _get_runner(host_tables)
    results, _ = run(in_maps)
    out = np.concatenate([r["out"] for r in results], axis=1)
    return np.ascontiguousarray(out.astype(np.float32)), results


def bench(in_maps, host_tables, iters=30, reps=1):
    """Return (best_per_exec_seconds, times list) by timing repeated dispatches."""
    import time
    import jax

    run = _get_runner(host_tables, reps)
    _, (sharded, concat_in, concat_zero) = run(in_maps)
    # device-resident inputs to avoid re-transfer
    din = [jax.device_put(a) for a in concat_in]
    dzero = [jax.device_put(a) for a in concat_zero]
    jax.block_until_ready(sharded(*din, *dzero))
    times = []
    for _ in range(iters):
        t0 = time.perf_counter()
        jax.block_until_ready(sharded(*din, *dzero))
        times.append(time.perf_counter() - t0)
    return min(times), times


def bench_device(in_maps, host_tables, reps=256, iters=10):
    """Estimate true per-kernel device time: (T_reps - T_1) / (reps - 1),
    cancelling the (dominant) axon dispatch overhead."""
    t1, _ = bench(in_maps, host_tables, iters=iters, reps=1)
    tr, _ = bench(in_maps, host_tables, iters=iters, reps=reps)
    return (tr - t1) / (reps - 1), t1, tr


def kernel(x, func_parameter, borders, inverse_chunk_lengths):
    in_maps, host_tables = _host_inputs(x, func_parameter, borders, inverse_chunk_lengths)
    out, _ = _run(in_maps, host_tables, trace=False)
    return out


def kernel_with_stats(x, func_parameter, borders, inverse_chunk_lengths, trace=True):
    """Returns (out, results) - test harness helper."""
    in_maps, host_tables = _host_inputs(x, func_parameter, borders, inverse_chunk_lengths)
    out, results = _run(in_maps, host_tables)
    return out, (in_maps, host_tables)


# revision 31
# speedup vs baseline: 3.1775x; 3.1775x over previous
"""Trainium2 Bass kernel for BasisFunction1D (piecewise-linear basis / histogram binning).

Math:
  out[o, b] = sum_i (1-d)*P[g, o, i] + d*P[g+1, o, i],
  g = bucket of x[i,b] on a Laplace-CDF grid, d = in-bucket linear position.

Key identity: with u = g + d the per-dim function is piecewise-linear in u with
knots at the integers, so using hat(v) = relu(v+1) - 2 relu(v) + relu(v-1):

  sum_g P_g * hat(u - g)  =  P_0 * (u+1)  +  sum_m Q_m * relu(u - m)

with Q_m = P_{m+1} - 2 P_m + P_{m-1} (second difference along g, built on host).
This needs ONE fused elementwise op per grid cell instead of the two-pass
|.| / min hat construction, and no per-input-dim PE broadcast at all.

fp32r matmul inputs are rounded to a ~10-bit mantissa, so raw relu(u-m) values
(up to 128) lose too much precision.  Two-level fix keeping every rhs in [0,8]:
with c~_q = min(u - 8q, 8)  (16 coarse tensors, one tensor_scalar each),

  relu(u - 8q - j) = max(c~_q - j, 0) + relu(u - 8(q+1))
  relu(u - 8k)     = sum_{k'>=k} max(c~_k', 0) + relu(u - 128)
  u                = sum_k max(c~_k, 0) + relu(u - 128) - relu(-u)

Folding the telescoped coarse sums into the host-side lhs table:

  out = C*1 + sum_{q,j} L_{q,j} * max(c~_q - j, 0)
        [+ OV * relu(u - 128) + RL * relu(-u)]   (zero on the closed-form path)

  L_{q,j} = Q_{8q+j} + (j==0) * (S'_q + P_0),  S'_q = sum_{q'<q} sum_j Q_{8q'+j}
  C = P_0,  OV = sum_m Q_m + P_0,  RL = P_0 - P_1

Every fine tile is ONE fused tensor_scalar / activation instruction, all rhs
values lie in [0,8] (exactly representable saturations; only the active
block's fractional value rounds), and all matmuls run fp32r at 1 cycle/row.

Device algorithm (per core, batch shard of 1024):
  1. Compute u[i,b] = g + d in closed form (the borders array is the
     inverse-Laplace-CDF grid; verified host-side, with an exact host-table
     fallback otherwise).
  2. ones tile (memset), then per q: c~_q on DVE, then 8 fine tiles
     round-robin on DVE / ACT / Pool so no single engine bottlenecks.
  3. out accumulates in PSUM over the matching fp32r matmuls against the
     host-side table [C, L_0..L_127, OV, RL].
"""

import math

import numpy as np

I_DIM = 128
O_DIM = 128
G = 128
B_FULL = 8192
N_CORES = 8
BS = B_FULL // N_CORES
NBLK = 131  # C(ones), L_0..L_127, OV, RL

_NC_CACHE = {}


def _ref_grid_f64():
    def inv(u):
        return math.log(2.0 * u) if u <= 0.5 else -math.log(2.0 * (1.0 - u))

    cs = 1.0 / G
    b = [inv(i * cs) for i in range(1, G)]
    left = b[0] - (b[1] - b[0])
    right = b[-1] + (b[-1] - b[-2])
    return np.array([left] + b + [right], dtype=np.float64)


def _grid_matches(borders, inv_len):
    ref = _ref_grid_f64()
    ref32 = ref.astype(np.float32)
    il_ref = (1.0 / (ref32[1:].astype(np.float64) - ref32[:-1].astype(np.float64))).astype(
        np.float32
    )
    return np.allclose(borders, ref32, rtol=1e-5, atol=1e-5) and np.allclose(
        inv_len, il_ref, rtol=1e-4, atol=1e-4
    )


def _build_nc(host_tables: bool, reps: int = 1):
    from contextlib import ExitStack

    import concourse.bacc as bacc
    import concourse.mybir as mybir
    import concourse.tile as tile

    dt = mybir.dt
    f32 = dt.float32
    f32r = dt.float32r
    AF = mybir.ActivationFunctionType
    OP = mybir.AluOpType

    nc = bacc.Bacc("TRN2", target_bir_lowering=False, debug=False)

    x_d = nc.dram_tensor("x", [I_DIM, BS], f32, kind="ExternalInput")
    # [i, j*o]: block j=0 is P_0^T (linear term), j=1..128 are Q_{j-1}^T,
    # j=129 is (2P_0 - P_1)^T (u<0 correction)
    qt_d = nc.dram_tensor("qt", [I_DIM, NBLK * O_DIM], f32r, kind="ExternalInput")
    # bias table: col m (0..127) = -m (ACT relu biases); col 128 = -64; col 129 = -63.5
    nb_d = nc.dram_tensor("nb", [128, 130], f32, kind="ExternalInput")
    if host_tables:
        hg_d = nc.dram_tensor("hg", [I_DIM, BS], f32, kind="ExternalInput")
        hb_d = nc.dram_tensor("hbor", [I_DIM, BS], f32, kind="ExternalInput")
        hi_d = nc.dram_tensor("hil", [I_DIM, BS], f32, kind="ExternalInput")
    out_d = nc.dram_tensor("out", [O_DIM, BS], f32, kind="ExternalOutput")

    with tile.TileContext(nc) as tc, ExitStack() as ctx:
        if reps > 1:
            loop_cm = tc.For_i(
                0,
                reps,
                1,
                hint_engines=(
                    mybir.EngineType.PE,
                    mybir.EngineType.Activation,
                    mybir.EngineType.DVE,
                ),
            )
            ctx.enter_context(loop_cm)
        pers = ctx.enter_context(tc.tile_pool(name="pers", bufs=1))
        scr = ctx.enter_context(tc.tile_pool(name="scr", bufs=1))
        rpool = ctx.enter_context(tc.tile_pool(name="rpool", bufs=10))
        cpool = ctx.enter_context(tc.tile_pool(name="cpool", bufs=6))
        opsum = ctx.enter_context(tc.tile_pool(name="opsum", bufs=1, space="PSUM"))

        x_sb = pers.tile([I_DIM, BS], f32, tag="x", name="x")
        nc.sync.dma_start(x_sb[:], x_d.ap())
        nb_sb = pers.tile([128, 130], f32, tag="nb", name="nb")
        nc.sync.dma_start(nb_sb[:], nb_d.ap())

        # Q table in 8 chunks so early matmuls don't wait on the full load
        NCH = 8
        qt_sb = pers.tile([I_DIM, NBLK * O_DIM], f32r, tag="qt", name="qt")
        per = (NBLK + NCH - 1) // NCH
        for c in range(NCH):
            lo = c * per * O_DIM
            hi = min(NBLK, (c + 1) * per) * O_DIM
            nc.sync.dma_start(qt_sb[:, lo:hi], qt_d.ap()[:, lo:hi])

        def sct(tag, dtype=f32):
            return scr.tile([I_DIM, BS], dtype, tag=tag, name=tag)

        u = pers.tile([I_DIM, BS], f32, tag="u", name="u")

        if not host_tables:
            # ---- closed-form u = g + (x - borders[g]) * inv_len[g] ----
            # processed in two column halves so ACT/DVE stages pipeline
            gf = pers.tile([I_DIM, BS], f32, tag="gf", name="gf")
            HB = BS // 2
            for h in range(2):
                cs = slice(h * HB, (h + 1) * HB)
                ax = sct(f"T0{h}")
                nc.scalar.activation(ax[:, cs], x_sb[:, cs], AF.Abs)
                e = sct(f"T1{h}")
                nc.scalar.activation(e[:, cs], ax[:, cs], AF.Exp, scale=-1.0)
                s = sct(f"T2{h}")
                nc.scalar.activation(s[:, cs], x_sb[:, cs], AF.Sign)
                se = sct(f"T0{h}")
                nc.gpsimd.tensor_mul(se[:, cs], s[:, cs], e[:, cs])
                t1 = sct(f"T1{h}")
                nc.vector.tensor_sub(t1[:, cs], s[:, cs], se[:, cs])
                # y = cdf * 128 = 64 + 64*s*(1-e)
                y = sct(f"T0{h}")
                nc.vector.tensor_scalar(y[:, cs], t1[:, cs], 64.0, 64.0, OP.mult, OP.add)
                gi = scr.tile([I_DIM, BS], dt.int32, tag=f"T3{h}", name=f"T3{h}")
                nc.vector.tensor_copy(gi[:, cs], y[:, cs])
                g0 = sct(f"T1{h}")
                nc.vector.tensor_copy(g0[:, cs], gi[:, cs])
                # robust floor regardless of the converter's rounding mode
                cg = sct(f"T4{h}")
                nc.vector.tensor_tensor(cg[:, cs], g0[:, cs], y[:, cs], op=OP.is_gt)
                gf1 = sct(f"T3{h}")
                nc.vector.tensor_sub(gf1[:, cs], g0[:, cs], cg[:, cs])
                nc.vector.tensor_scalar(gf[:, cs], gf1[:, cs], 0.0, 127.0, OP.max, OP.min)
                # borders[g] = sign * ln(m1/64), m1 = max(min(g, 128-g), 0.5)
                a1 = sct(f"T1{h}")
                nc.scalar.activation(a1[:, cs], gf[:, cs], AF.Abs, bias=nb_sb[:, 128:129])
                a1c = sct(f"T3{h}")
                nc.vector.tensor_scalar(a1c[:, cs], a1[:, cs], 63.5, None, OP.min)
                L = sct(f"T1{h}")
                nc.scalar.activation(L[:, cs], a1c[:, cs], AF.Ln, scale=-1.0 / 64.0, bias=1.0)
                sL = sct(f"T3{h}")
                nc.gpsimd.tensor_mul(sL[:, cs], s[:, cs], L[:, cs])
                xb = sct(f"T2{h}")  # x - borders[g]  (= x + s*L)
                nc.gpsimd.tensor_add(xb[:, cs], x_sb[:, cs], sL[:, cs])
                # inv_len[g] = 1/ln(1 + 1/m2), m2 = max(min(g, 127-g), 1)
                a2 = sct(f"T0{h}")
                nc.scalar.activation(a2[:, cs], gf[:, cs], AF.Abs, bias=nb_sb[:, 129:130])
                nm2 = sct(f"T1{h}")  # -m2
                nc.vector.tensor_scalar(nm2[:, cs], a2[:, cs], 63.5, -1.0, OP.subtract, OP.min)
                rm = sct(f"T0{h}")  # -1/m2
                nc.vector.reciprocal_approx_fast(rm[:, cs], nm2[:, cs])
                q = sct(f"T1{h}")  # ln(1 + 1/m2)
                nc.scalar.activation(q[:, cs], rm[:, cs], AF.Ln, scale=-1.0, bias=1.0)
                il = sct(f"T0{h}")
                nc.vector.reciprocal_approx_fast(il[:, cs], q[:, cs])
                d_ = sct(f"T1{h}")
                nc.vector.tensor_mul(d_[:, cs], xb[:, cs], il[:, cs])
                nc.vector.tensor_add(u[:, cs], gf[:, cs], d_[:, cs])
        else:
            hg_sb = pers.tile([I_DIM, BS], f32, tag="hg", name="hg")
            nc.sync.dma_start(hg_sb[:], hg_d.ap())
            hb_sb = sct("T0")
            nc.sync.dma_start(hb_sb[:], hb_d.ap())
            hi_sb = sct("T1")
            nc.sync.dma_start(hi_sb[:], hi_d.ap())
            xb = sct("T2")
            nc.vector.tensor_sub(xb[:], x_sb[:], hb_sb[:])
            d_ = sct("T0")
            nc.vector.tensor_mul(d_[:], xb[:], hi_sb[:])
            nc.vector.tensor_add(u[:], hg_sb[:], d_[:])

        # ---- main loop: rhs tile then its two accumulate matmuls ----
        # block 0: ones; blocks 1+8q+j: max(c~_q - j, 0); 129: relu(u-128),
        # 130: relu(-u).  Fine producers round-robin DVE/ACT (gpsimd
        # tensor_scalar is uncalibrated software); consecutive blocks
        # alternate between two PSUM regions so back-to-back accumulates
        # never target the same bank; the two regions are summed at the end.
        HALF = BS // 2
        acc0 = opsum.tile([O_DIM, HALF], f32, tag="acc0", name="acc0")
        acc1 = opsum.tile([O_DIM, HALF], f32, tag="acc1", name="acc1")

        def mm_pair(blk, t):
            # column halves go to different PSUM bank regions so consecutive
            # matmuls never accumulate into the same bank back-to-back
            first = blk == 0
            last = blk == NBLK - 1
            lhsT = qt_sb[:, blk * O_DIM : (blk + 1) * O_DIM]
            nc.tensor.matmul(
                acc0[:, :], lhsT, t[:, 0:HALF],
                start=first, stop=last, skip_group_check=True,
            )
            nc.tensor.matmul(
                acc1[:, :], lhsT, t[:, HALF:BS],
                start=first, stop=last, skip_group_check=True,
            )

        ones = rpool.tile([I_DIM, BS], f32r, tag="r", name="r")
        nc.vector.tensor_scalar(ones[:], x_sb[:], 0.0, 1.0, OP.mult, OP.add)
        mm_pair(0, ones)

        # deterministic weighted assignment of the 128 fine tiles
        # (gpsimd tensor_scalar measured ~12us/tile on HW - never use it)
        share = {"D": 80.0 / 128, "A": 48.0 / 128}
        used = {"D": 0, "A": 0}
        for q in range(16):
            ct = cpool.tile([I_DIM, BS], f32, tag="c", name="c")
            nc.vector.tensor_scalar(ct[:], u[:], 8.0 * q, 8.0, OP.subtract, OP.min)
            for j in range(8):
                f = 8 * q + j
                eng = max("DA", key=lambda e: share[e] * (f + 1) - used[e])
                used[eng] += 1
                t = rpool.tile([I_DIM, BS], f32r, tag="r", name="r")
                if eng == "A":
                    nc.scalar.activation(t[:], ct[:], AF.Relu, bias=nb_sb[:, j : j + 1])
                else:
                    nc.vector.tensor_scalar(t[:], ct[:], float(j), 0.0, OP.subtract, OP.max)
                mm_pair(1 + f, t)
        # u < 0 and u > 128 occur even on the closed-form path (the two
        # extrapolation buckets reach past the borders array)
        ov = rpool.tile([I_DIM, BS], f32r, tag="r", name="r")
        nc.vector.tensor_scalar(ov[:], u[:], 128.0, 0.0, OP.subtract, OP.max)
        mm_pair(129, ov)
        rl = rpool.tile([I_DIM, BS], f32r, tag="r", name="r")
        nc.vector.tensor_scalar(rl[:], u[:], -1.0, 0.0, OP.mult, OP.max)
        mm_pair(130, rl)

        out_sb = pers.tile([O_DIM, BS], f32, tag="osb", name="osb")
        nc.vector.tensor_copy(out_sb[:, 0:HALF], acc0[:, :])
        nc.scalar.copy(out_sb[:, HALF:BS], acc1[:, :])
        nc.sync.dma_start(out_d.ap(), out_sb[:])

    return nc


def _get_nc(host_tables: bool, reps: int = 1):
    key = (bool(host_tables), reps)
    if key not in _NC_CACHE:
        _NC_CACHE[key] = _build_nc(key[0], reps)
    return _NC_CACHE[key]


def _host_inputs(x, func_parameter, borders, inverse_chunk_lengths):
    x = np.ascontiguousarray(np.asarray(x, dtype=np.float32))
    P = np.asarray(func_parameter, dtype=np.float32)
    borders = np.asarray(borders, dtype=np.float32)
    inv_len = np.asarray(inverse_chunk_lengths, dtype=np.float32)

    host_tables = not _grid_matches(borders, inv_len)

    # blocks in use order: C(ones), L_0..L_127, OV, RL   (each [O, I])
    Pd = P.astype(np.float64)
    Qm = Pd[1 : G + 1] - 2.0 * Pd[0:G]
    Qm[1:] += Pd[0 : G - 1]  # Q_m = P_{m+1} - 2 P_m + P_{m-1}
    R = Qm.reshape(16, 8, O_DIM, I_DIM).sum(axis=1)  # [16, O, I]
    Sp = np.cumsum(R, axis=0) - R  # S'_q = sum_{q'<q} R_{q'}
    L = Qm.reshape(16, 8, O_DIM, I_DIM).copy()
    L[:, 0] += Sp + Pd[0]
    blocks = np.concatenate(
        [
            Pd[0:1],  # C
            L.reshape(G, O_DIM, I_DIM),
            (Qm.sum(axis=0) + Pd[0])[None],  # OV
            (Pd[0] - Pd[1])[None],  # RL
        ],
        axis=0,
    ).astype(np.float32)  # [131, O, I]
    qt = np.ascontiguousarray(blocks.transpose(2, 0, 1).reshape(I_DIM, NBLK * O_DIM))

    nb = np.zeros((128, 130), dtype=np.float32)
    nb[:, 0:128] = -np.arange(128, dtype=np.float32)[None, :]
    nb[:, 128] = -64.0
    nb[:, 129] = -63.5

    in_maps = []
    for c in range(N_CORES):
        xs = np.ascontiguousarray(x[:, c * BS : (c + 1) * BS])
        m = {"x": xs, "qt": qt, "nb": nb}
        if host_tables:
            # exact fallback: bucketize host-side with the provided tables
            exp_na = np.exp(-np.abs(xs))
            cdf = np.where(xs > 0, 1.0 - 0.5 * exp_na, 0.5 * exp_na).astype(np.float32)
            idx = np.clip((cdf * G).astype(np.int32), 0, G - 1)
            m["hg"] = idx.astype(np.float32)
            m["hbor"] = borders[idx].astype(np.float32)
            m["hil"] = inv_len[idx].astype(np.float32)
        in_maps.append(m)
    return in_maps, host_tables


_RUNNER_CACHE = {}


def _get_runner(host_tables, reps: int = 1):
    """Cached jitted 8-core runner (mirrors bass2jax.run_bass_via_pjrt multi-core path)."""
    key = (bool(host_tables), reps)
    if key in _RUNNER_CACHE:
        return _RUNNER_CACHE[key]

    import jax
    from jax.sharding import Mesh, PartitionSpec
    from jax.experimental.shard_map import shard_map
    import concourse.mybir as mybir
    from concourse.bass2jax import (
        _bass_exec_p,
        install_neuronx_cc_hook,
        partition_id_tensor,
    )

    install_neuronx_cc_hook()
    nc = _get_nc(host_tables, reps)
    if not nc.is_finalized():
        nc.finalize()
    assert nc.dbg_addr is None
    partition_name = nc.partition_id_tensor.name if nc.partition_id_tensor else None

    in_names, out_names, out_avals, zero_outs = [], [], [], []
    for alloc in nc.m.functions[0].allocations:
        if not isinstance(alloc, mybir.MemoryLocationSet):
            continue
        name = alloc.memorylocations[0].name
        if alloc.kind == "ExternalInput":
            if name != partition_name:
                in_names.append(name)
        elif alloc.kind == "ExternalOutput":
            shape = tuple(alloc.tensor_shape)
            dtype = mybir.dt.np(alloc.dtype)
            out_names.append(name)
            out_avals.append(jax.core.ShapedArray(shape, dtype))
            zero_outs.append(np.zeros(shape, dtype))
    n_params = len(in_names)
    all_names = in_names + out_names
    if partition_name is not None:
        all_names = all_names + [partition_name]

    def _body(*args):
        operands = list(args)
        if partition_name is not None:
            operands.append(partition_id_tensor())
        outs = _bass_exec_p.bind(
            *operands,
            out_avals=tuple(out_avals),
            in_names=tuple(all_names),
            out_names=tuple(out_names),
            lowering_input_output_aliases=(),
            sim_require_finite=True,
            sim_require_nnan=True,
            nc=nc,
        )
        return tuple(outs)

    devices = jax.devices()[:N_CORES]
    mesh = Mesh(np.asarray(devices), ("core",))
    n_outs = len(out_names)
    sharded = jax.jit(
        shard_map(
            _body,
            mesh=mesh,
            in_specs=(PartitionSpec("core"),) * (n_params + n_outs),
            out_specs=(PartitionSpec("core"),) * n_outs,
            check_rep=False,
        ),
        keep_unused=True,
    )

    def run(in_maps):
        concat_in = [
            np.concatenate([np.asarray(m[name]) for m in in_maps], axis=0)
            for name in in_names
        ]
        concat_zero = [
            np.zeros((N_CORES * z.shape[0], *z.shape[1:]), z.dtype) for z in zero_outs
        ]
        out_arrs = sharded(*concat_in, *concat_zero)
        res = [
            {
                name: np.asarray(out_arrs[i]).reshape(N_CORES, *out_avals[i].shape)[c]
                for i, name in enumerate(out_names)
            }
            for c in range(N_CORES)
        ]
        return res, (sharded, concat_in, concat_zero)

    _RUNNER_CACHE[key] = run
    return run


def _run(in_maps, host_tables, trace=False):
    run = _get_runner(host_tables)
    results, _ = run(in_maps)
    out = np.concatenate([r["out"] for r in results], axis=1)
    return np.ascontiguousarray(out.astype(np.float32)), results


def bench(in_maps, host_tables, iters=30, reps=1):
    """Return (best_per_exec_seconds, times list) by timing repeated dispatches."""
    import time
    import jax

    run = _get_runner(host_tables, reps)
    _, (sharded, concat_in, concat_zero) = run(in_maps)
    # device-resident inputs to avoid re-transfer
    din = [jax.device_put(a) for a in concat_in]
    dzero = [jax.device_put(a) for a in concat_zero]
    jax.block_until_ready(sharded(*din, *dzero))
    times = []
    for _ in range(iters):
        t0 = time.perf_counter()
        jax.block_until_ready(sharded(*din, *dzero))
        times.append(time.perf_counter() - t0)
    return min(times), times


def bench_device(in_maps, host_tables, reps=256, iters=10):
    """Estimate true per-kernel device time: (T_reps - T_1) / (reps - 1),
    cancelling the (dominant) axon dispatch overhead."""
    t1, _ = bench(in_maps, host_tables, iters=iters, reps=1)
    tr, _ = bench(in_maps, host_tables, iters=iters, reps=reps)
    return (tr - t1) / (reps - 1), t1, tr


def kernel(x, func_parameter, borders, inverse_chunk_lengths):
    in_maps, host_tables = _host_inputs(x, func_parameter, borders, inverse_chunk_lengths)
    out, _ = _run(in_maps, host_tables, trace=False)
    return out


def kernel_with_stats(x, func_parameter, borders, inverse_chunk_lengths, trace=True):
    """Returns (out, results) - test harness helper."""
    in_maps, host_tables = _host_inputs(x, func_parameter, borders, inverse_chunk_lengths)
    out, results = _run(in_maps, host_tables)
    return out, (in_maps, host_tables)
